# revision 3
# baseline (speedup 1.0000x reference)
"""Depthwise 4D conv (3,3,3,3) kernel for Trainium2, 8 NeuronCores — v9.

3D-Toeplitz matmul over (y, z, t) with x-taps as 3 PSUM-shifted passes
(see kernel_v3 docstring for the core mapping). v9 halves the input DMA
traffic: instead of reloading the full 4-y-row window per y0 block, each
channel keeps a persistent SBUF tile whose partition groups are y-rows
mod 4; stepping y0 -> y0+1 only DMAs the 2 new y-rows (256KB) over the 2
expired groups. Odd y0 windows see their rows at partition groups
rotated by 2, handled by a second (row-permuted) copy of the weights.
Two channel-chains interleave so each chain's update DMA hides under the
other chain's matmuls.
"""

import os
import sys

import numpy as np

for _p in ("/opt/trn_rl_repo",):
    if _p not in sys.path and os.path.isdir(_p):
        sys.path.insert(0, _p)

B, C, X, Y, Z, T = 4, 64, 32, 32, 32, 8
N_CORES = 8
CH = C // N_CORES                          # 8 channels per core
YP, ZP = Y + 2, Z + 2                      # padded extents
NBX = B * X                                # 128 free cols per z-instance
ROW = 32 * 16 * NBX                        # elems per y-row panel (65536)

LAST_EXEC_NS = None


def _build_wts(kernel_np: np.ndarray) -> np.ndarray:
    """kernel [81, C, 1] -> lhsT [C, 2, 128, 3*32] (f32), parity variants.

    Parity 0 (even y0): lhsT[c,0,krow=(yw,zw,tw),k1*32+m=(ly,lz,to)]
    = w4[k1, yw-ly, zw-lz, tw-to+1, c] where valid. Parity 1: partition
    row-groups rotated so group g holds window row (g+2)%4.
    """
    w4 = kernel_np.reshape(3, 3, 3, 3, C).astype(np.float32)
    wt = np.zeros((C, 128, 96), np.float32)
    for ly in range(2):
        for lz in range(2):
            for to in range(T):
                m = ly * 16 + lz * 8 + to
                for k2 in range(3):
                    yw = ly + k2
                    for k3 in range(3):
                        zw = lz + k3
                        for k4 in range(3):
                            tw = to + k4 - 1
                            if 0 <= tw < T:
                                krow = yw * 32 + zw * 8 + tw
                                for k1 in range(3):
                                    wt[:, krow, k1 * 32 + m] = w4[k1, k2, k3, k4]
    wt4 = wt.reshape(C, 4, 32, 96)
    wt_odd = np.roll(wt4, -2, axis=1).reshape(C, 128, 96)
    return np.stack([wt, wt_odd], axis=1)  # [C, 2, 128, 96]


_NC_CACHE: dict = {}


def _get_nc(repeats: int = 1):
    key = ("nc", repeats)
    if key in _NC_CACHE:
        return _NC_CACHE[key]

    import concourse.mybir as mybir
    from concourse import bacc
    from concourse.bass import AP
    from concourse.tile import TileContext

    f32 = mybir.dt.float32
    bf16 = mybir.dt.bfloat16
    nc = bacc.Bacc("TRN2", target_bir_lowering=False, debug=False,
                   num_devices=N_CORES)

    # per-y-row z-expanded panels: [ci, yp, (zw,tw)=32, slot=16, (b,x)=128]
    xrows = nc.dram_tensor("xrows", (CH * YP * ROW,), bf16,
                           kind="ExternalInput").ap()
    wts = nc.dram_tensor("wts", (128, CH * 192), bf16,
                         kind="ExternalInput").ap()
    # out[c, y0, p=(zg, ly, lz, to), (zj, b, x)]
    out = nc.dram_tensor("out", (CH, 16, 128, 4 * NBX), bf16,
                         kind="ExternalOutput").ap()

    def fill(tile, ci, yp0, part0, nrows):
        src = AP(xrows.tensor, (ci * YP + yp0) * ROW,
                 [[16 * NBX, 32 * nrows], [1, 16 * NBX]])
        nc.sync.dma_start(
            out=tile[32 * part0 : 32 * (part0 + nrows)], in_=src)

    with TileContext(nc) as tc:
        with tc.tile_pool(name="w", bufs=1) as wpool, \
             tc.tile_pool(name="in", bufs=8) as ipool, \
             tc.tile_pool(name="out", bufs=6) as opool, \
             tc.tile_pool(name="ps", bufs=6, space="PSUM") as pspool:
            wtile = wpool.tile([128, CH * 192], bf16, tag="w")
            nc.sync.dma_start(out=wtile[:], in_=wts)
            for _ in range(repeats):
                for cp in range(CH // 4):
                    cis = tuple(4 * cp + i for i in range(4))
                    tiles = []
                    for ci in cis:
                        t = ipool.tile([128, 16 * NBX], bf16, tag="in")
                        fill(t, ci, 0, 0, 4)
                        tiles.append(t)
                    for y0 in range(16):
                        par = y0 % 2
                        for s, ci in enumerate(cis):
                            itile = tiles[s]
                            ptile = pspool.tile([128, 4 * NBX], f32,
                                                tag="ps")
                            for k1 in (1, 0, 2):
                                wsl = wtile[:, ci * 192 + par * 96
                                            + k1 * 32
                                            : ci * 192 + par * 96
                                            + k1 * 32 + 32]
                                for zg in range(4):
                                    pblk = ptile[32 * zg : 32 * zg + 32]
                                    rblk = itile[:, zg * 4 * NBX
                                                 : (zg + 1) * 4 * NBX]
                                    if k1 == 1:
                                        o_ap, r_ap = pblk, rblk
                                    else:
                                        pv = pblk.rearrange(
                                            "p (j b x) -> p (j b) x",
                                            j=4, x=X)
                                        rv = rblk.rearrange(
                                            "p (j b x) -> p (j b) x",
                                            j=4, x=X)
                                        if k1 == 0:   # out x = xin + 1
                                            o_ap = pv[:, :, 1:X]
                                            r_ap = rv[:, :, 0 : X - 1]
                                        else:         # out x = xin - 1
                                            o_ap = pv[:, :, 0 : X - 1]
                                            r_ap = rv[:, :, 1:X]
                                    nc.tensor.matmul(
                                        o_ap, lhsT=wsl, rhs=r_ap,
                                        start=(k1 == 1), stop=(k1 == 2),
                                        tile_position=(0, 32 * zg),
                                        skip_group_check=(k1 != 1),
                                    )
                            if y0 < 15:
                                # replace the 2 expired row-groups with
                                # the rows y0+1 needs: rows 2*y0+4, +5 at
                                # groups (2*y0)%4 (+1)
                                fill(itile, ci, 2 * y0 + 4,
                                     (2 * y0) % 4, 2)
                            otile = opool.tile([128, 4 * NBX], bf16,
                                               tag="out")
                            nc.vector.tensor_copy(out=otile[:],
                                                  in_=ptile[:])
                            dst = AP(out.tensor,
                                     (ci * 16 + y0) * 4 * 128 * NBX,
                                     [[4 * NBX, 128], [1, 4 * NBX]])
                            nc.scalar.dma_start(out=dst, in_=otile[:])

    nc.finalize()
    _NC_CACHE[key] = nc
    return nc


def _get_runner_r(repeats: int):
    key = ("runner", repeats)
    if key in _NC_CACHE:
        return _NC_CACHE[key]

    import jax
    import concourse.mybir as mybir
    from concourse import bass2jax
    from concourse.bass2jax import _bass_exec_p, install_neuronx_cc_hook
    from jax.experimental.shard_map import shard_map
    from jax.sharding import Mesh, NamedSharding, PartitionSpec

    nc = _get_nc(repeats)
    install_neuronx_cc_hook()

    partition_name = (
        nc.partition_id_tensor.name if nc.partition_id_tensor else None
    )
    in_names, out_names, out_avals, zero_outs = [], [], [], []
    for alloc in nc.m.functions[0].allocations:
        if not isinstance(alloc, mybir.MemoryLocationSet):
            continue
        name = alloc.memorylocations[0].name
        if alloc.kind == "ExternalInput":
            if name != partition_name:
                in_names.append(name)
        elif alloc.kind == "ExternalOutput":
            shape = tuple(alloc.tensor_shape)
            dtype = mybir.dt.np(alloc.dtype)
            out_names.append(name)
            out_avals.append(jax.core.ShapedArray(shape, dtype))
            zero_outs.append(np.zeros(shape, dtype))
    n_params = len(in_names)
    all_in_names = list(in_names) + list(out_names)
    if partition_name is not None:
        all_in_names.append(partition_name)

    def _body(*args):
        operands = list(args)
        if partition_name is not None:
            operands.append(bass2jax.partition_id_tensor())
        outs = _bass_exec_p.bind(
            *operands,
            out_avals=tuple(out_avals),
            in_names=tuple(all_in_names),
            out_names=tuple(out_names),
            lowering_input_output_aliases=(),
            sim_require_finite=True,
            sim_require_nnan=True,
            nc=nc,
        )
        return tuple(outs)

    devices = jax.devices()[:N_CORES]
    mesh = Mesh(np.asarray(devices), ("core",))
    spec = PartitionSpec("core")
    n_args = n_params + len(out_names)
    sharded = jax.jit(
        shard_map(_body, mesh=mesh, in_specs=(spec,) * n_args,
                  out_specs=(spec,) * len(out_names), check_rep=False),
        keep_unused=True,
    )
    sharding = NamedSharding(mesh, spec)

    def run(in_maps, timing_reps=0):
        concat_in = [
            np.concatenate([np.asarray(in_maps[c][name])
                            for c in range(N_CORES)], axis=0)
            for name in in_names
        ]
        concat_zero = [
            np.zeros((N_CORES * z.shape[0], *z.shape[1:]), z.dtype)
            for z in zero_outs
        ]
        dev_args = [jax.device_put(a, sharding)
                    for a in (*concat_in, *concat_zero)]
        out_arrs = jax.block_until_ready(sharded(*dev_args))

        exec_ns = None
        if timing_reps > 0:
            import time
            sharded(*dev_args)  # extra warmup
            jax.block_until_ready(sharded(*dev_args))
            t0 = time.perf_counter()
            for _ in range(timing_reps):
                last = sharded(*dev_args)
            jax.block_until_ready(last)
            exec_ns = (time.perf_counter() - t0) / (timing_reps * repeats) * 1e9

        results = [
            {name: np.asarray(out_arrs[i]).reshape(
                N_CORES, *out_avals[i].shape)[c]
             for i, name in enumerate(out_names)}
            for c in range(N_CORES)
        ]
        return results, exec_ns

    _NC_CACHE[key] = run
    return run


def _make_in_maps(x, w):
    import ml_dtypes
    from numpy.lib.stride_tricks import sliding_window_view
    bf16 = ml_dtypes.bfloat16
    wt = _build_wts(w)  # [C, 2, 128, 96]
    in_maps = []
    for k in range(N_CORES):
        c0 = k * CH
        src = x[:, c0 : c0 + CH]           # [B, CH, X, Y, Z, T]
        arr = np.zeros((CH, YP, ZP, T, B, X), bf16)
        arr[:, 1 : Y + 1, 1 : Z + 1] = src.transpose(1, 3, 4, 5, 0, 2)
        # per-row z-expanded panels: [c, yp, (zw, tw), slot, (b, x)]
        swz = sliding_window_view(arr, (4,), axis=(2,))
        swz = swz[:, :, 0 : 2 * 16 : 2]    # (c, yp, slot, t, b, x, zw)
        xr = np.ascontiguousarray(
            swz.transpose(0, 1, 6, 3, 2, 4, 5)
        )  # (c, yp, zw, t, slot, b, x)
        # [CH, 2, 128, 96] -> [128, CH*192]
        wc = np.ascontiguousarray(
            wt[c0 : c0 + CH].transpose(2, 0, 1, 3)
        ).reshape(128, CH * 192).astype(bf16)
        in_maps.append({"xrows": xr.ravel(), "wts": wc})
    return in_maps


R_TIME = 8


def kernel(inputs, kernel, _timing_reps=0):
    global LAST_EXEC_NS
    x = np.asarray(inputs, dtype=np.float32)
    w = np.asarray(kernel, dtype=np.float32)
    assert x.shape == (B, C, X, Y, Z, T), x.shape
    assert w.shape == (81, C, 1), w.shape

    run = _get_runner_r(R_TIME)
    results, exec_ns = run(_make_in_maps(x, w), timing_reps=_timing_reps)
    LAST_EXEC_NS = exec_ns

    full = np.empty((B, C, X, Y, Z, T), np.float32)
    for k in range(N_CORES):
        c0 = k * CH
        o = results[k]["out"].reshape(CH, 16, 4, 2, 2, T, 4, B, X)
        # dims: (c, y0, zg, ly, lz, to, zj, b, x) ->
        #       (b, c, x, y0, ly, zg, zj, lz, to)
        full[:, c0 : c0 + CH] = o.transpose(
            7, 0, 8, 1, 3, 2, 6, 4, 5
        ).reshape(B, CH, X, Y, Z, T).astype(np.float32)
    return full


# revision 4
# speedup vs baseline: 1.0100x; 1.0100x over previous
"""Depthwise 4D conv (3,3,3,3) kernel for Trainium2, 8 NeuronCores — v9.

3D-Toeplitz matmul over (y, z, t) with x-taps as 3 PSUM-shifted passes
(see kernel_v3 docstring for the core mapping). v9 halves the input DMA
traffic: instead of reloading the full 4-y-row window per y0 block, each
channel keeps a persistent SBUF tile whose partition groups are y-rows
mod 4; stepping y0 -> y0+1 only DMAs the 2 new y-rows (256KB) over the 2
expired groups. Odd y0 windows see their rows at partition groups
rotated by 2, handled by a second (row-permuted) copy of the weights.
Two channel-chains interleave so each chain's update DMA hides under the
other chain's matmuls.
"""

import os
import sys

import numpy as np

for _p in ("/opt/trn_rl_repo",):
    if _p not in sys.path and os.path.isdir(_p):
        sys.path.insert(0, _p)

B, C, X, Y, Z, T = 4, 64, 32, 32, 32, 8
N_CORES = 8
CH = C // N_CORES                          # 8 channels per core
YP, ZP = Y + 2, Z + 2                      # padded extents
NBX = B * X                                # 128 free cols per z-instance
ROW = 32 * 16 * NBX                        # elems per y-row panel (65536)

LAST_EXEC_NS = None


def _build_wts(kernel_np: np.ndarray) -> np.ndarray:
    """kernel [81, C, 1] -> lhsT [C, 2, 128, 3*32] (f32), parity variants.

    Parity 0 (even y0): lhsT[c,0,krow=(yw,zw,tw),k1*32+m=(ly,lz,to)]
    = w4[k1, yw-ly, zw-lz, tw-to+1, c] where valid. Parity 1: partition
    row-groups rotated so group g holds window row (g+2)%4.
    """
    w4 = kernel_np.reshape(3, 3, 3, 3, C).astype(np.float32)
    wt = np.zeros((C, 128, 96), np.float32)
    for ly in range(2):
        for lz in range(2):
            for to in range(T):
                m = ly * 16 + lz * 8 + to
                for k2 in range(3):
                    yw = ly + k2
                    for k3 in range(3):
                        zw = lz + k3
                        for k4 in range(3):
                            tw = to + k4 - 1
                            if 0 <= tw < T:
                                krow = yw * 32 + zw * 8 + tw
                                for k1 in range(3):
                                    wt[:, krow, k1 * 32 + m] = w4[k1, k2, k3, k4]
    wt4 = wt.reshape(C, 4, 32, 96)
    wt_odd = np.roll(wt4, -2, axis=1).reshape(C, 128, 96)
    return np.stack([wt, wt_odd], axis=1)  # [C, 2, 128, 96]


_NC_CACHE: dict = {}


def _get_nc(repeats: int = 1):
    key = ("nc", repeats)
    if key in _NC_CACHE:
        return _NC_CACHE[key]

    import concourse.mybir as mybir
    from concourse import bacc
    from concourse.bass import AP
    from concourse.tile import TileContext

    f32 = mybir.dt.float32
    bf16 = mybir.dt.bfloat16
    nc = bacc.Bacc("TRN2", target_bir_lowering=False, debug=False,
                   num_devices=N_CORES)

    # per-y-row z-expanded panels: [ci, yp, (zw,tw)=32, slot=16, (b,x)=128]
    xrows = nc.dram_tensor("xrows", (CH * YP * ROW,), bf16,
                           kind="ExternalInput").ap()
    wts = nc.dram_tensor("wts", (128, CH * 192), bf16,
                         kind="ExternalInput").ap()
    # out[c, y0, p=(zg, ly, lz, to), (zj, b, x)]
    out = nc.dram_tensor("out", (CH, 16, 128, 4 * NBX), bf16,
                         kind="ExternalOutput").ap()

    def fill(tile, ci, yp0, part0, nrows):
        src = AP(xrows.tensor, (ci * YP + yp0) * ROW,
                 [[16 * NBX, 32 * nrows], [1, 16 * NBX]])
        nc.sync.dma_start(
            out=tile[32 * part0 : 32 * (part0 + nrows)], in_=src)

    with TileContext(nc) as tc:
        with tc.tile_pool(name="w", bufs=1) as wpool, \
             tc.tile_pool(name="in", bufs=10) as ipool, \
             tc.tile_pool(name="out", bufs=6) as opool, \
             tc.tile_pool(name="ps", bufs=6, space="PSUM") as pspool:
            wtile = wpool.tile([128, CH * 192], bf16, tag="w")
            nc.sync.dma_start(out=wtile[:], in_=wts)
            for _ in range(repeats):
                for cp in range(CH // 8):
                    cis = tuple(8 * cp + i for i in range(8))
                    tiles = []
                    for ci in cis:
                        t = ipool.tile([128, 16 * NBX], bf16, tag="in")
                        fill(t, ci, 0, 0, 4)
                        tiles.append(t)
                    for y0 in range(16):
                        par = y0 % 2
                        for s, ci in enumerate(cis):
                            itile = tiles[s]
                            ptile = pspool.tile([128, 4 * NBX], f32,
                                                tag="ps")
                            for k1 in (1, 0, 2):
                                wsl = wtile[:, ci * 192 + par * 96
                                            + k1 * 32
                                            : ci * 192 + par * 96
                                            + k1 * 32 + 32]
                                for zg in range(4):
                                    pblk = ptile[32 * zg : 32 * zg + 32]
                                    rblk = itile[:, zg * 4 * NBX
                                                 : (zg + 1) * 4 * NBX]
                                    if k1 == 1:
                                        o_ap, r_ap = pblk, rblk
                                    else:
                                        pv = pblk.rearrange(
                                            "p (j b x) -> p (j b) x",
                                            j=4, x=X)
                                        rv = rblk.rearrange(
                                            "p (j b x) -> p (j b) x",
                                            j=4, x=X)
                                        if k1 == 0:   # out x = xin + 1
                                            o_ap = pv[:, :, 1:X]
                                            r_ap = rv[:, :, 0 : X - 1]
                                        else:         # out x = xin - 1
                                            o_ap = pv[:, :, 0 : X - 1]
                                            r_ap = rv[:, :, 1:X]
                                    nc.tensor.matmul(
                                        o_ap, lhsT=wsl, rhs=r_ap,
                                        start=(k1 == 1), stop=(k1 == 2),
                                        tile_position=(0, 32 * zg),
                                        skip_group_check=(k1 != 1),
                                    )
                            if y0 < 15:
                                # replace the 2 expired row-groups with
                                # the rows y0+1 needs: rows 2*y0+4, +5 at
                                # groups (2*y0)%4 (+1)
                                fill(itile, ci, 2 * y0 + 4,
                                     (2 * y0) % 4, 2)
                            otile = opool.tile([128, 4 * NBX], bf16,
                                               tag="out")
                            nc.vector.tensor_copy(out=otile[:],
                                                  in_=ptile[:])
                            dst = AP(out.tensor,
                                     (ci * 16 + y0) * 4 * 128 * NBX,
                                     [[4 * NBX, 128], [1, 4 * NBX]])
                            nc.scalar.dma_start(out=dst, in_=otile[:])

    nc.finalize()
    _NC_CACHE[key] = nc
    return nc


def _get_runner_r(repeats: int):
    key = ("runner", repeats)
    if key in _NC_CACHE:
        return _NC_CACHE[key]

    import jax
    import concourse.mybir as mybir
    from concourse import bass2jax
    from concourse.bass2jax import _bass_exec_p, install_neuronx_cc_hook
    from jax.experimental.shard_map import shard_map
    from jax.sharding import Mesh, NamedSharding, PartitionSpec

    nc = _get_nc(repeats)
    install_neuronx_cc_hook()

    partition_name = (
        nc.partition_id_tensor.name if nc.partition_id_tensor else None
    )
    in_names, out_names, out_avals, zero_outs = [], [], [], []
    for alloc in nc.m.functions[0].allocations:
        if not isinstance(alloc, mybir.MemoryLocationSet):
            continue
        name = alloc.memorylocations[0].name
        if alloc.kind == "ExternalInput":
            if name != partition_name:
                in_names.append(name)
        elif alloc.kind == "ExternalOutput":
            shape = tuple(alloc.tensor_shape)
            dtype = mybir.dt.np(alloc.dtype)
            out_names.append(name)
            out_avals.append(jax.core.ShapedArray(shape, dtype))
            zero_outs.append(np.zeros(shape, dtype))
    n_params = len(in_names)
    all_in_names = list(in_names) + list(out_names)
    if partition_name is not None:
        all_in_names.append(partition_name)

    def _body(*args):
        operands = list(args)
        if partition_name is not None:
            operands.append(bass2jax.partition_id_tensor())
        outs = _bass_exec_p.bind(
            *operands,
            out_avals=tuple(out_avals),
            in_names=tuple(all_in_names),
            out_names=tuple(out_names),
            lowering_input_output_aliases=(),
            sim_require_finite=True,
            sim_require_nnan=True,
            nc=nc,
        )
        return tuple(outs)

    devices = jax.devices()[:N_CORES]
    mesh = Mesh(np.asarray(devices), ("core",))
    spec = PartitionSpec("core")
    n_args = n_params + len(out_names)
    sharded = jax.jit(
        shard_map(_body, mesh=mesh, in_specs=(spec,) * n_args,
                  out_specs=(spec,) * len(out_names), check_rep=False),
        keep_unused=True,
    )
    sharding = NamedSharding(mesh, spec)

    def run(in_maps, timing_reps=0):
        concat_in = [
            np.concatenate([np.asarray(in_maps[c][name])
                            for c in range(N_CORES)], axis=0)
            for name in in_names
        ]
        concat_zero = [
            np.zeros((N_CORES * z.shape[0], *z.shape[1:]), z.dtype)
            for z in zero_outs
        ]
        dev_args = [jax.device_put(a, sharding)
                    for a in (*concat_in, *concat_zero)]
        out_arrs = jax.block_until_ready(sharded(*dev_args))

        exec_ns = None
        if timing_reps > 0:
            import time
            sharded(*dev_args)  # extra warmup
            jax.block_until_ready(sharded(*dev_args))
            t0 = time.perf_counter()
            for _ in range(timing_reps):
                last = sharded(*dev_args)
            jax.block_until_ready(last)
            exec_ns = (time.perf_counter() - t0) / (timing_reps * repeats) * 1e9

        results = [
            {name: np.asarray(out_arrs[i]).reshape(
                N_CORES, *out_avals[i].shape)[c]
             for i, name in enumerate(out_names)}
            for c in range(N_CORES)
        ]
        return results, exec_ns

    _NC_CACHE[key] = run
    return run


def _make_in_maps(x, w):
    import ml_dtypes
    from numpy.lib.stride_tricks import sliding_window_view
    bf16 = ml_dtypes.bfloat16
    wt = _build_wts(w)  # [C, 2, 128, 96]
    in_maps = []
    for k in range(N_CORES):
        c0 = k * CH
        src = x[:, c0 : c0 + CH]           # [B, CH, X, Y, Z, T]
        arr = np.zeros((CH, YP, ZP, T, B, X), bf16)
        arr[:, 1 : Y + 1, 1 : Z + 1] = src.transpose(1, 3, 4, 5, 0, 2)
        # per-row z-expanded panels: [c, yp, (zw, tw), slot, (b, x)]
        swz = sliding_window_view(arr, (4,), axis=(2,))
        swz = swz[:, :, 0 : 2 * 16 : 2]    # (c, yp, slot, t, b, x, zw)
        xr = np.ascontiguousarray(
            swz.transpose(0, 1, 6, 3, 2, 4, 5)
        )  # (c, yp, zw, t, slot, b, x)
        # [CH, 2, 128, 96] -> [128, CH*192]
        wc = np.ascontiguousarray(
            wt[c0 : c0 + CH].transpose(2, 0, 1, 3)
        ).reshape(128, CH * 192).astype(bf16)
        in_maps.append({"xrows": xr.ravel(), "wts": wc})
    return in_maps


R_TIME = 8


def kernel(inputs, kernel, _timing_reps=0):
    global LAST_EXEC_NS
    x = np.asarray(inputs, dtype=np.float32)
    w = np.asarray(kernel, dtype=np.float32)
    assert x.shape == (B, C, X, Y, Z, T), x.shape
    assert w.shape == (81, C, 1), w.shape

    run = _get_runner_r(R_TIME)
    results, exec_ns = run(_make_in_maps(x, w), timing_reps=_timing_reps)
    LAST_EXEC_NS = exec_ns

    full = np.empty((B, C, X, Y, Z, T), np.float32)
    for k in range(N_CORES):
        c0 = k * CH
        o = results[k]["out"].reshape(CH, 16, 4, 2, 2, T, 4, B, X)
        # dims: (c, y0, zg, ly, lz, to, zj, b, x) ->
        #       (b, c, x, y0, ly, zg, zj, lz, to)
        full[:, c0 : c0 + CH] = o.transpose(
            7, 0, 8, 1, 3, 2, 6, 4, 5
        ).reshape(B, CH, X, Y, Z, T).astype(np.float32)
    return full
